# revision 25
# baseline (speedup 1.0000x reference)
"""Multi-head attention block (qkv -> attention -> o_net -> residual+LN) on
8 Trainium2 NeuronCores.

Problem (hardcoded): B=2, T=2048, D=1024, H=16, dh=64, fp32 I/O.
Reference quirk: the (B,H,T,dh) attention buffer is viewed as (H,B,T,dh)
before the output projection, i.e. output batch b2 / head-slot h2 takes the
attention output of original (b, h) with 16*b + h == 2*h2 + b2.

Sharding: tensor-parallel by head ("pair" f = 16*b + h). Core c owns pairs
f in {4c..4c+3}, i.e. batch bc = c//4, heads 4*(c%4)..4*(c%4)+3, and runs
qkv projection + attention for those pairs over ALL 2048 query positions.
No startup collective: every core receives the full (replicated) input.
At the end an AllToAll (one per duo, 0.5 MB each) redistributes attention
outputs by query position so core c applies o_net + residual + layernorm to
its own output slice: positions [c*256,(c+1)*256) of BOTH output batches.
Core c's pairs map to o_net column blocks h2 in {2c, 2c+1} (b2 = f % 2).
"""
import sys
sys.path.insert(0, "/opt/trn_rl_repo")
import contextlib
import numpy as np
import ml_dtypes

import concourse.bass as bass
from concourse import bacc
import concourse.mybir as mybir
import concourse.tile as tile
from concourse.bass_utils import run_bass_kernel_spmd

BF16 = mybir.dt.bfloat16
F32 = mybir.dt.float32
nbf16 = ml_dtypes.bfloat16

N_CORES = 8
B, T, D = 2, 2048, 1024
H, DH = 16, 64
TC = T // N_CORES          # 256 output positions per core (per batch)
NTOK = B * TC              # 512 output tokens per core (both batches)
LN_EPS = 1e-5

A2A_BLK = 128 * TC         # per-destination block of one duo's A2A (elems)

_prog_cache = {}


def _build_program(reps=1):
    import os as _os
    KCC = int(_os.environ.get("KCC", "1"))  # 2: A2A/duo, 1: one A2A, 0: none
    KSKIP = _os.environ.get("KSKIP", "")    # timing-only ablations: att,qkv,onet
    nc = bacc.Bacc("TRN2", num_devices=N_CORES)

    # ---- per-core inputs (host pre-tiled / pre-transposed) ----
    xT = nc.dram_tensor("xT", [128, 8, T], BF16, kind="ExternalInput")
    inp_res = nc.dram_tensor("inp_res", [NTOK, D], F32, kind="ExternalInput")
    wqT = nc.dram_tensor("wqT", [128, 8, 256], BF16, kind="ExternalInput")
    wkT = nc.dram_tensor("wkT", [128, 8, 256], BF16, kind="ExternalInput")
    wvT = nc.dram_tensor("wvT", [128, 8, 256], BF16, kind="ExternalInput")
    woT = nc.dram_tensor("woT", [128, 8, 1024], BF16, kind="ExternalInput")
    b_q = nc.dram_tensor("b_q", [1, 256], BF16, kind="ExternalInput")
    b_k = nc.dram_tensor("b_k", [1, 256], BF16, kind="ExternalInput")
    b_v = nc.dram_tensor("b_v", [1, 256], BF16, kind="ExternalInput")
    onesd = nc.dram_tensor("onesd", [1, T], BF16, kind="ExternalInput")
    gamma = nc.dram_tensor("gamma", [1, D], F32, kind="ExternalInput")
    beta = nc.dram_tensor("beta", [1, D], F32, kind="ExternalInput")

    out = nc.dram_tensor("out", [NTOK, D], F32, kind="ExternalOutput")

    def bcast_rows(src_row_ap, nrows):
        return bass.AP(tensor=src_row_ap.tensor, offset=src_row_ap.offset,
                       ap=[[0, nrows]] + src_row_ap.ap[1:])

    with tile.TileContext(nc) as tc:
        with contextlib.ExitStack() as ctx:
            dram = ctx.enter_context(tc.tile_pool(name="dram", bufs=1, space="DRAM"))
            dram_sc = ctx.enter_context(tc.tile_pool(name="dram_sc", bufs=4, space="DRAM"))
            cst = ctx.enter_context(tc.tile_pool(name="cst", bufs=1))

            if KCC == 1:
                a2a_in = [dram.tile([N_CORES * 2 * A2A_BLK], BF16, name="a2a_in0")]
                a2a_out1 = dram.tile([N_CORES, 2 * A2A_BLK], BF16, name="a2a_out1")
            else:
                a2a_in = [dram.tile([N_CORES * A2A_BLK], BF16, name=f"a2a_in{i}")
                          for i in range(2)]
                a2a_out = [dram.tile([N_CORES, A2A_BLK], BF16, name=f"a2a_out{i}")
                           for i in range(2)]

            ones_sb = cst.tile([1, T], BF16)
            nc.sync.dma_start(out=ones_sb[:], in_=onesd[:])
            bq_sb = cst.tile([1, 256], BF16)
            nc.sync.dma_start(out=bq_sb[:], in_=b_q[:])
            bk_sb = cst.tile([1, 256], BF16)
            nc.sync.dma_start(out=bk_sb[:], in_=b_k[:])
            bv_sb = cst.tile([1, 256], BF16)
            nc.sync.dma_start(out=bv_sb[:], in_=b_v[:])

            # on-chip qkv state (live through attention)
            qT_sb = cst.tile([128, 2, T], BF16)     # chunk j: pair-2j ch 0:64, 2j+1 ch 64:128
            kT_sb = cst.tile([128, 2, T], BF16)
            v_sb = cst.tile([128, 16, 4, DH + 1], BF16)   # [key128, kc, pair, ch+one]
            attnv = cst.tile([128, 2, T], BF16)     # [ch(2 pairs), duo, query]

            # ---------------- phase 1: qkv projection -----------------
            with tc.tile_pool(name="projw", bufs=1) as projw, \
                 tc.tile_pool(name="psproj", bufs=4, space="PSUM") as psproj:
              if "qkv" in KSKIP:
                nc.vector.memset(qT_sb[:], 0.001)
                nc.vector.memset(kT_sb[:], 0.001)
                nc.vector.memset(v_sb[:], 0.001)
              else:
                xT_sb = projw.tile([128, 8, T], BF16)
                wq_sb = projw.tile([128, 8, 256], BF16)
                wk_sb = projw.tile([128, 8, 256], BF16)
                wv_sb = projw.tile([128, 8, 256], BF16)
                for dt in range(8):
                    nc.sync.dma_start(out=xT_sb[:, dt, :], in_=xT[:, dt, :])
                    nc.sync.dma_start(out=wk_sb[:, dt, :], in_=wkT[:, dt, :])
                    nc.sync.dma_start(out=wq_sb[:, dt, :], in_=wqT[:, dt, :])
                    nc.sync.dma_start(out=wv_sb[:, dt, :], in_=wvT[:, dt, :])
                nc.vector.memset(v_sb[:, :, :, DH:DH + 1], 1.0)

                # K then Q, duo-chunk 0 first so attention can start early
                for (dst, w_sb, bias) in ((kT_sb, wk_sb, bk_sb), (qT_sb, wq_sb, bq_sb)):
                    for j in range(2):
                        for tch in range(4):
                            pp = psproj.tile([128, 512], F32, tag="pp")
                            nc.tensor.matmul(out=pp[:], lhsT=bias[0:1, j * 128:(j + 1) * 128],
                                             rhs=ones_sb[0:1, tch * 512:(tch + 1) * 512],
                                             start=True, stop=False)
                            for dt in range(8):
                                nc.tensor.matmul(out=pp[:], lhsT=w_sb[:, dt, j * 128:(j + 1) * 128],
                                                 rhs=xT_sb[:, dt, tch * 512:(tch + 1) * 512],
                                                 start=False, stop=(dt == 7))
                            nc.vector.tensor_copy(out=dst[:, j, tch * 512:(tch + 1) * 512],
                                                  in_=pp[:])

                # V key-major: [128 keys, 4 pairs x 64] per key chunk
                for tch in range(16):
                    pv = psproj.tile([128, 256], F32, tag="pv")
                    nc.tensor.matmul(out=pv[:], lhsT=ones_sb[0:1, 0:128],
                                     rhs=bv_sb[0:1, :], start=True, stop=False)
                    for dt in range(8):
                        nc.tensor.matmul(out=pv[:], lhsT=xT_sb[:, dt, tch * 128:(tch + 1) * 128],
                                         rhs=wv_sb[:, dt, :], start=False, stop=(dt == 7))
                    nc.vector.tensor_copy(
                        out=v_sb[:, tch, :, 0:DH],
                        in_=pv[:].rearrange("p (l d) -> p l d", d=DH))

            # phase-4 constants: issue loads now so they overlap attention
            wo_sb = cst.tile([128, 8, 1024], BF16)
            nc.sync.dma_start(out=wo_sb[:], in_=woT[:])
            res_sb = cst.tile([128, 4, D], F32)
            nc.sync.dma_start(out=res_sb[:],
                              in_=inp_res.rearrange("(c p) d -> p c d", p=128))
            gb_sb = cst.tile([128, D], F32)
            nc.gpsimd.dma_start(out=gb_sb[:], in_=bcast_rows(gamma[0:1, :], 128))
            bb_sb = cst.tile([128, D], F32)
            nc.gpsimd.dma_start(out=bb_sb[:], in_=bcast_rows(beta[0:1, :], 128))
            eps_sb = cst.tile([128, 1], F32)
            nc.vector.memset(eps_sb[:], LN_EPS)
            att_in = cst.tile([128, 8, 2, TC], BF16)

            # ---------------- phase 2: attention (2 duos) --------------
            if "att" in KSKIP:
                nc.vector.memset(attnv[:], 0.001)
                for dd in range(2):
                    for d in range(N_CORES):
                        if KCC == 1:
                            dst = a2a_in[0][(d * 2 + dd) * A2A_BLK:
                                            (d * 2 + dd + 1) * A2A_BLK]
                        else:
                            dst = a2a_in[dd][d * A2A_BLK:(d + 1) * A2A_BLK]
                        nc.sync.dma_start(
                            out=dst.rearrange("(p f) -> p f", p=128),
                            in_=attnv[:, dd, d * TC:(d + 1) * TC])
                    if KCC == 2:
                        nc.gpsimd.collective_compute(
                            "AllToAll", mybir.AluOpType.bypass,
                            replica_groups=[list(range(N_CORES))],
                            ins=[a2a_in[dd][:]], outs=[a2a_out[dd][:]],
                        )
                    elif KCC == 0:
                        nc.gpsimd.dma_start(
                            out=a2a_out[dd][:].rearrange("r f -> (r f)"),
                            in_=a2a_in[dd][:])
                    elif KCC == 1 and dd == 1:
                        nc.gpsimd.collective_compute(
                            "AllToAll", mybir.AluOpType.bypass,
                            replica_groups=[list(range(N_CORES))],
                            ins=[a2a_in[0][:]], outs=[a2a_out1[:]],
                        )
            else:
             for _rep in range(reps):
              with tc.tile_pool(name="pt", bufs=3) as ptp, \
                 tc.tile_pool(name="nrm", bufs=3) as nrm, \
                 tc.tile_pool(name="pss", bufs=2, space="PSUM") as pss, \
                 tc.tile_pool(name="psb", bufs=1, space="PSUM") as psb, \
                 tc.tile_pool(name="pso", bufs=1, space="PSUM") as pso:
                for dd in range(2):
                    for qc in range(4):
                        poA = pso.tile([65, 512], F32, tag="poA")
                        poB = pso.tile([65, 512], F32, tag="poB")
                        for kc in range(16):
                            pscr = pss.tile([128, 1024], F32, tag="pscr")
                            nc.tensor.matmul(out=pscr[:, 0:512],
                                             lhsT=kT_sb[0:64, dd, kc * 128:(kc + 1) * 128],
                                             rhs=qT_sb[0:64, dd, qc * 512:(qc + 1) * 512],
                                             start=True, stop=True, tile_position=(0, 0))
                            nc.tensor.matmul(out=pscr[:, 512:1024],
                                             lhsT=kT_sb[64:128, dd, kc * 128:(kc + 1) * 128],
                                             rhs=qT_sb[64:128, dd, qc * 512:(qc + 1) * 512],
                                             start=True, stop=True, tile_position=(64, 0))
                            pt = ptp.tile([128, 1024], BF16, tag="pt")
                            nc.scalar.activation(out=pt[:], in_=pscr[:],
                                                 func=mybir.ActivationFunctionType.Exp,
                                                 scale=0.125)
                            nc.tensor.matmul(out=poA[:], lhsT=v_sb[:, kc, 2 * dd, :],
                                             rhs=pt[:, 0:512],
                                             start=(kc == 0), stop=(kc == 15))
                            nc.tensor.matmul(out=poB[:], lhsT=v_sb[:, kc, 2 * dd + 1, :],
                                             rhs=pt[:, 512:1024],
                                             start=(kc == 0), stop=(kc == 15))

                        # normalize: evacuate PSUM, divide by the ones-row sum
                        oA = nrm.tile([65, 512], F32, tag="oA")
                        nc.vector.tensor_copy(out=oA[:], in_=poA[:])
                        oB = nrm.tile([65, 512], F32, tag="oB")
                        nc.vector.tensor_copy(out=oB[:], in_=poB[:])
                        recA = nrm.tile([1, 512], BF16, tag="recA")
                        recB = nrm.tile([1, 512], BF16, tag="recB")
                        with nc.allow_low_precision(reason="1/denom feeds bf16 attnv"):
                            nc.vector.reciprocal(out=recA[:], in_=oA[64:65, :])
                            nc.vector.reciprocal(out=recB[:], in_=oB[64:65, :])
                        # broadcast 1/denom to 64 rows via a 1-row PE matmul
                        rbA = psb.tile([64, 512], F32, tag="rbA")
                        nc.tensor.matmul(out=rbA[:], lhsT=ones_sb[0:1, 0:64],
                                         rhs=recA[:], start=True, stop=True)
                        rbB = psb.tile([64, 512], F32, tag="rbB")
                        nc.tensor.matmul(out=rbB[:], lhsT=ones_sb[0:1, 0:64],
                                         rhs=recB[:], start=True, stop=True)
                        nc.vector.tensor_tensor(out=attnv[0:64, dd, qc * 512:(qc + 1) * 512],
                                                in0=oA[0:64, :], in1=rbA[:],
                                                op=mybir.AluOpType.mult)
                        nc.vector.tensor_tensor(out=attnv[64:128, dd, qc * 512:(qc + 1) * 512],
                                                in0=oB[0:64, :], in1=rbB[:],
                                                op=mybir.AluOpType.mult)

                    if _rep == reps - 1:
                        # ship this duo's attention output: block d holds
                        # [128 ch, 256 query] for destination core d
                        for d in range(N_CORES):
                            if KCC == 1:
                                dst = a2a_in[0][(d * 2 + dd) * A2A_BLK:
                                                (d * 2 + dd + 1) * A2A_BLK]
                            else:
                                dst = a2a_in[dd][d * A2A_BLK:(d + 1) * A2A_BLK]
                            nc.sync.dma_start(
                                out=dst.rearrange("(p f) -> p f", p=128),
                                in_=attnv[:, dd, d * TC:(d + 1) * TC])
                        if KCC == 2:
                            nc.gpsimd.collective_compute(
                                "AllToAll", mybir.AluOpType.bypass,
                                replica_groups=[list(range(N_CORES))],
                                ins=[a2a_in[dd][:]], outs=[a2a_out[dd][:]],
                            )
                        elif KCC == 0:
                            nc.gpsimd.dma_start(
                                out=a2a_out[dd][:].rearrange("r f -> (r f)"),
                                in_=a2a_in[dd][:])
                        elif KCC == 1 and dd == 1:
                            nc.gpsimd.collective_compute(
                                "AllToAll", mybir.AluOpType.bypass,
                                replica_groups=[list(range(N_CORES))],
                                ins=[a2a_in[0][:]], outs=[a2a_out1[:]],
                            )

            # ---------------- phase 4: o_net + residual + layernorm ----
            with tc.tile_pool(name="fin", bufs=2) as fin, \
                 tc.tile_pool(name="psf", bufs=4, space="PSUM") as psf:
                # gather A2A results: att_in[p, s, b2, t], p = (h2-2s)*64 + ch
                for dd in range(2):
                    for b2 in range(2):
                        if KCC == 1:
                            src = bass.AP(
                                tensor=a2a_out1.tensor,
                                offset=a2a_out1.offset + dd * A2A_BLK + b2 * 64 * TC,
                                ap=[[TC, 64], [2 * A2A_BLK, 8], [1, TC]])
                        else:
                            src = bass.AP(
                                tensor=a2a_out[dd].tensor,
                                offset=a2a_out[dd].offset + b2 * 64 * TC,
                                ap=[[TC, 64], [A2A_BLK, 8], [1, TC]])
                        nc.sync.dma_start(out=att_in[dd * 64:(dd + 1) * 64, :, b2, :],
                                          in_=src)

                for b2 in range(2):
                    for tch in range(2):
                        chunk = b2 * 2 + tch
                        x = fin.tile([128, D], F32, tag="x")
                        for nn_ in range(2):
                            po = psf.tile([128, 512], F32, tag="po")
                            for s in range(8):
                                nc.tensor.matmul(
                                    out=po[:],
                                    lhsT=att_in[:, s, b2, tch * 128:(tch + 1) * 128],
                                    rhs=wo_sb[:, s, nn_ * 512:(nn_ + 1) * 512],
                                    start=(s == 0), stop=(s == 7))
                            nc.vector.tensor_tensor(out=x[:, nn_ * 512:(nn_ + 1) * 512],
                                                    in0=po[:],
                                                    in1=res_sb[:, chunk, nn_ * 512:(nn_ + 1) * 512],
                                                    op=mybir.AluOpType.add)
                        stats = fin.tile([128, 2, 6], F32, tag="stats")
                        for s2 in range(2):
                            nc.vector.bn_stats(out=stats[:, s2, :], in_=x[:, s2 * 512:(s2 + 1) * 512])
                        mv = fin.tile([128, 2], F32, tag="mv")
                        nc.vector.bn_aggr(out=mv[:], in_=stats[:])
                        sd = fin.tile([128, 1], F32, tag="sd")
                        nc.scalar.activation(out=sd[:], in_=mv[:, 1:2],
                                             func=mybir.ActivationFunctionType.Sqrt,
                                             bias=eps_sb[:], scale=1.0)
                        rstd = fin.tile([128, 1], F32, tag="rstd")
                        nc.vector.reciprocal(out=rstd[:], in_=sd[:])
                        y = fin.tile([128, D], F32, tag="y")
                        nc.vector.tensor_scalar(out=y[:], in0=x[:],
                                                scalar1=mv[:, 0:1], scalar2=rstd[:],
                                                op0=mybir.AluOpType.subtract,
                                                op1=mybir.AluOpType.mult)
                        yg = fin.tile([128, D], F32, tag="yg")
                        nc.gpsimd.tensor_tensor(out=yg[:], in0=y[:], in1=gb_sb[:],
                                                op=mybir.AluOpType.mult)
                        yb = fin.tile([128, D], F32, tag="yb")
                        nc.gpsimd.tensor_tensor(out=yb[:], in0=yg[:], in1=bb_sb[:],
                                                op=mybir.AluOpType.add)
                        nc.sync.dma_start(
                            out=out[chunk * 128:(chunk + 1) * 128, :], in_=yb[:])

    nc.finalize()
    return nc


def _get_program(reps=1):
    import os as _os
    key = (reps, _os.environ.get("KCC", "1"), _os.environ.get("KSKIP", ""))
    if key not in _prog_cache:
        _prog_cache[key] = _build_program(reps)
    return _prog_cache[key]


def _prep_inputs(inp, W_qkv, b_qkv, W_o, gamma, beta):
    """Build the 8 per-core input dicts (host-side)."""
    f32 = np.float32
    inp = np.asarray(inp, f32)
    W_qkv = np.asarray(W_qkv, f32)
    b_qkv = np.asarray(b_qkv, f32)
    W_o = np.asarray(W_o, f32)
    gamma = np.asarray(gamma, f32).reshape(1, D)
    beta = np.asarray(beta, f32).reshape(1, D)

    woT = np.ascontiguousarray(
        W_o.T.reshape(8, 128, 1024).transpose(1, 0, 2)).astype(nbf16)
    ones = np.ones((1, T), nbf16)

    xT_b = []
    for b in range(B):
        xT_b.append(np.ascontiguousarray(
            inp[b].T.reshape(8, 128, T).transpose(1, 0, 2)).astype(nbf16))

    in_maps = []
    for c in range(N_CORES):
        bc = c // 4
        r0 = 4 * (c % 4) * 64            # first channel row of this core's heads
        rows = slice(r0, r0 + 256)
        wqT = np.ascontiguousarray(
            W_qkv[0:1024][rows].T.reshape(8, 128, 256).transpose(1, 0, 2)).astype(nbf16)
        wkT = np.ascontiguousarray(
            W_qkv[1024:2048][rows].T.reshape(8, 128, 256).transpose(1, 0, 2)).astype(nbf16)
        wvT = np.ascontiguousarray(
            W_qkv[2048:3072][rows].T.reshape(8, 128, 256).transpose(1, 0, 2)).astype(nbf16)
        sl = slice(c * TC, (c + 1) * TC)
        x_res = np.concatenate([inp[0, sl, :], inp[1, sl, :]], axis=0)
        in_maps.append({
            "xT": xT_b[bc],
            "inp_res": np.ascontiguousarray(x_res),
            "wqT": wqT, "wkT": wkT, "wvT": wvT, "woT": woT,
            "b_q": b_qkv[0:1024][rows].reshape(1, 256).astype(nbf16),
            "b_k": b_qkv[1024:2048][rows].reshape(1, 256).astype(nbf16),
            "b_v": b_qkv[2048:3072][rows].reshape(1, 256).astype(nbf16),
            "onesd": ones, "gamma": gamma, "beta": beta,
        })
    return in_maps


def _assemble(results):
    out = np.empty((B, T, D), np.float32)
    for c in range(N_CORES):
        o = results[c]["out"]
        sl = slice(c * TC, (c + 1) * TC)
        out[0, sl, :] = o[0:TC, :]
        out[1, sl, :] = o[TC:NTOK, :]
    return out


def kernel(inp, W_qkv, b_qkv, W_o, gamma, beta):
    nc = _get_program()
    in_maps = _prep_inputs(inp, W_qkv, b_qkv, W_o, gamma, beta)
    res = run_bass_kernel_spmd(nc, in_maps, core_ids=list(range(N_CORES)))
    return _assemble(res.results)


if __name__ == "__main__":
    rng = np.random.RandomState(0)
    inp = rng.randn(B, T, D).astype(np.float32)
    W_qkv = (rng.randn(3 * H * DH, D) * D ** -0.5).astype(np.float32)
    b_qkv = (rng.randn(3 * H * DH) * 0.02).astype(np.float32)
    W_o = (rng.randn(D, H * DH) * (H * DH) ** -0.5).astype(np.float32)
    gamma = np.ones(D, np.float32)
    beta = np.zeros(D, np.float32)
    out = kernel(inp=inp, W_qkv=W_qkv, b_qkv=b_qkv, W_o=W_o, gamma=gamma, beta=beta)
    print("out", out.shape, out.dtype, np.abs(out).mean())


# revision 28
# speedup vs baseline: 1.0053x; 1.0053x over previous
"""Multi-head attention block (qkv -> attention -> o_net -> residual+LN) on
8 Trainium2 NeuronCores.

Problem (hardcoded): B=2, T=2048, D=1024, H=16, dh=64, fp32 I/O.
Reference quirk: the (B,H,T,dh) attention buffer is viewed as (H,B,T,dh)
before the output projection, i.e. output batch b2 / head-slot h2 takes the
attention output of original (b, h) with 16*b + h == 2*h2 + b2.

Sharding: tensor-parallel by head ("pair" f = 16*b + h). Core c owns pairs
f in {4c..4c+3}, i.e. batch bc = c//4, heads 4*(c%4)..4*(c%4)+3, and runs
qkv projection + attention for those pairs over ALL 2048 query positions.
No startup collective: every core receives the full (replicated) input.
At the end an AllToAll (one per duo, 0.5 MB each) redistributes attention
outputs by query position so core c applies o_net + residual + layernorm to
its own output slice: positions [c*256,(c+1)*256) of BOTH output batches.
Core c's pairs map to o_net column blocks h2 in {2c, 2c+1} (b2 = f % 2).
"""
import sys
sys.path.insert(0, "/opt/trn_rl_repo")
import contextlib
import numpy as np
import ml_dtypes

import concourse.bass as bass
from concourse import bacc
import concourse.mybir as mybir
import concourse.tile as tile
from concourse.bass_utils import run_bass_kernel_spmd

BF16 = mybir.dt.bfloat16
F32 = mybir.dt.float32
nbf16 = ml_dtypes.bfloat16

N_CORES = 8
B, T, D = 2, 2048, 1024
H, DH = 16, 64
TC = T // N_CORES          # 256 output positions per core (per batch)
NTOK = B * TC              # 512 output tokens per core (both batches)
LN_EPS = 1e-5

A2A_BLK = 128 * TC         # per-destination block of one duo's A2A (elems)

_prog_cache = {}


def _build_program(reps=1):
    KCC = 1       # single merged AllToAll (measured best vs per-duo A2As)
    KSKIP = ""    # build everything (ablation scaffolding lives in kernel_knobs.py)
    nc = bacc.Bacc("TRN2", num_devices=N_CORES)

    # ---- per-core inputs (host pre-tiled / pre-transposed) ----
    xT = nc.dram_tensor("xT", [128, 8, T], BF16, kind="ExternalInput")
    inp_res = nc.dram_tensor("inp_res", [NTOK, D], F32, kind="ExternalInput")
    wqT = nc.dram_tensor("wqT", [128, 8, 256], BF16, kind="ExternalInput")
    wkT = nc.dram_tensor("wkT", [128, 8, 256], BF16, kind="ExternalInput")
    wvT = nc.dram_tensor("wvT", [128, 8, 256], BF16, kind="ExternalInput")
    woT = nc.dram_tensor("woT", [128, 8, 1024], BF16, kind="ExternalInput")
    b_q = nc.dram_tensor("b_q", [1, 256], BF16, kind="ExternalInput")
    b_k = nc.dram_tensor("b_k", [1, 256], BF16, kind="ExternalInput")
    b_v = nc.dram_tensor("b_v", [1, 256], BF16, kind="ExternalInput")
    onesd = nc.dram_tensor("onesd", [1, T], BF16, kind="ExternalInput")
    gamma = nc.dram_tensor("gamma", [1, D], F32, kind="ExternalInput")
    beta = nc.dram_tensor("beta", [1, D], F32, kind="ExternalInput")

    out = nc.dram_tensor("out", [NTOK, D], F32, kind="ExternalOutput")

    def bcast_rows(src_row_ap, nrows):
        return bass.AP(tensor=src_row_ap.tensor, offset=src_row_ap.offset,
                       ap=[[0, nrows]] + src_row_ap.ap[1:])

    with tile.TileContext(nc) as tc:
        with contextlib.ExitStack() as ctx:
            dram = ctx.enter_context(tc.tile_pool(name="dram", bufs=1, space="DRAM"))
            dram_sc = ctx.enter_context(tc.tile_pool(name="dram_sc", bufs=4, space="DRAM"))
            cst = ctx.enter_context(tc.tile_pool(name="cst", bufs=1))

            if KCC == 1:
                a2a_in = [dram.tile([N_CORES * 2 * A2A_BLK], BF16, name="a2a_in0")]
                a2a_out1 = dram.tile([N_CORES, 2 * A2A_BLK], BF16, name="a2a_out1")
            else:
                a2a_in = [dram.tile([N_CORES * A2A_BLK], BF16, name=f"a2a_in{i}")
                          for i in range(2)]
                a2a_out = [dram.tile([N_CORES, A2A_BLK], BF16, name=f"a2a_out{i}")
                           for i in range(2)]

            ones_sb = cst.tile([1, T], BF16)
            nc.sync.dma_start(out=ones_sb[:], in_=onesd[:])
            bq_sb = cst.tile([1, 256], BF16)
            nc.sync.dma_start(out=bq_sb[:], in_=b_q[:])
            bk_sb = cst.tile([1, 256], BF16)
            nc.sync.dma_start(out=bk_sb[:], in_=b_k[:])
            bv_sb = cst.tile([1, 256], BF16)
            nc.sync.dma_start(out=bv_sb[:], in_=b_v[:])

            # on-chip qkv state (live through attention)
            qT_sb = cst.tile([128, 2, T], BF16)     # chunk j: pair-2j ch 0:64, 2j+1 ch 64:128
            kT_sb = cst.tile([128, 2, T], BF16)
            v_sb = cst.tile([128, 16, 4, DH + 1], BF16)   # [key128, kc, pair, ch+one]
            attnv = cst.tile([128, 2, T], BF16)     # [ch(2 pairs), duo, query]

            # ---------------- phase 1: qkv projection -----------------
            with tc.tile_pool(name="projw", bufs=1) as projw, \
                 tc.tile_pool(name="psproj", bufs=4, space="PSUM") as psproj:
              if "qkv" in KSKIP:
                nc.vector.memset(qT_sb[:], 0.001)
                nc.vector.memset(kT_sb[:], 0.001)
                nc.vector.memset(v_sb[:], 0.001)
              else:
                xT_sb = projw.tile([128, 8, T], BF16)
                wq_sb = projw.tile([128, 8, 256], BF16)
                wk_sb = projw.tile([128, 8, 256], BF16)
                wv_sb = projw.tile([128, 8, 256], BF16)
                for dt in range(8):
                    nc.sync.dma_start(out=xT_sb[:, dt, :], in_=xT[:, dt, :])
                    nc.sync.dma_start(out=wk_sb[:, dt, :], in_=wkT[:, dt, :])
                    nc.sync.dma_start(out=wq_sb[:, dt, :], in_=wqT[:, dt, :])
                    nc.sync.dma_start(out=wv_sb[:, dt, :], in_=wvT[:, dt, :])
                nc.vector.memset(v_sb[:, :, :, DH:DH + 1], 1.0)

                # K then Q, duo-chunk 0 first so attention can start early
                for (dst, w_sb, bias) in ((kT_sb, wk_sb, bk_sb), (qT_sb, wq_sb, bq_sb)):
                    for j in range(2):
                        for tch in range(4):
                            pp = psproj.tile([128, 512], F32, tag="pp")
                            nc.tensor.matmul(out=pp[:], lhsT=bias[0:1, j * 128:(j + 1) * 128],
                                             rhs=ones_sb[0:1, tch * 512:(tch + 1) * 512],
                                             start=True, stop=False)
                            for dt in range(8):
                                nc.tensor.matmul(out=pp[:], lhsT=w_sb[:, dt, j * 128:(j + 1) * 128],
                                                 rhs=xT_sb[:, dt, tch * 512:(tch + 1) * 512],
                                                 start=False, stop=(dt == 7))
                            nc.vector.tensor_copy(out=dst[:, j, tch * 512:(tch + 1) * 512],
                                                  in_=pp[:])

                # V key-major: [128 keys, 4 pairs x 64] per key chunk
                for tch in range(16):
                    pv = psproj.tile([128, 256], F32, tag="pv")
                    nc.tensor.matmul(out=pv[:], lhsT=ones_sb[0:1, 0:128],
                                     rhs=bv_sb[0:1, :], start=True, stop=False)
                    for dt in range(8):
                        nc.tensor.matmul(out=pv[:], lhsT=xT_sb[:, dt, tch * 128:(tch + 1) * 128],
                                         rhs=wv_sb[:, dt, :], start=False, stop=(dt == 7))
                    nc.vector.tensor_copy(
                        out=v_sb[:, tch, :, 0:DH],
                        in_=pv[:].rearrange("p (l d) -> p l d", d=DH))

            # phase-4 constants: issue loads now so they overlap attention
            wo_sb = cst.tile([128, 8, 1024], BF16)
            nc.sync.dma_start(out=wo_sb[:], in_=woT[:])
            res_sb = cst.tile([128, 4, D], F32)
            nc.sync.dma_start(out=res_sb[:],
                              in_=inp_res.rearrange("(c p) d -> p c d", p=128))
            gb_sb = cst.tile([128, D], F32)
            nc.gpsimd.dma_start(out=gb_sb[:], in_=bcast_rows(gamma[0:1, :], 128))
            bb_sb = cst.tile([128, D], F32)
            nc.gpsimd.dma_start(out=bb_sb[:], in_=bcast_rows(beta[0:1, :], 128))
            eps_sb = cst.tile([128, 1], F32)
            nc.vector.memset(eps_sb[:], LN_EPS)
            att_in = cst.tile([128, 8, 2, TC], BF16)

            # ---------------- phase 2: attention (2 duos) --------------
            if "att" in KSKIP:
                nc.vector.memset(attnv[:], 0.001)
                for dd in range(2):
                    for d in range(N_CORES):
                        if KCC == 1:
                            dst = a2a_in[0][(d * 2 + dd) * A2A_BLK:
                                            (d * 2 + dd + 1) * A2A_BLK]
                        else:
                            dst = a2a_in[dd][d * A2A_BLK:(d + 1) * A2A_BLK]
                        nc.sync.dma_start(
                            out=dst.rearrange("(p f) -> p f", p=128),
                            in_=attnv[:, dd, d * TC:(d + 1) * TC])
                    if KCC == 2:
                        nc.gpsimd.collective_compute(
                            "AllToAll", mybir.AluOpType.bypass,
                            replica_groups=[list(range(N_CORES))],
                            ins=[a2a_in[dd][:]], outs=[a2a_out[dd][:]],
                        )
                    elif KCC == 0:
                        nc.gpsimd.dma_start(
                            out=a2a_out[dd][:].rearrange("r f -> (r f)"),
                            in_=a2a_in[dd][:])
                    elif KCC == 1 and dd == 1:
                        nc.gpsimd.collective_compute(
                            "AllToAll", mybir.AluOpType.bypass,
                            replica_groups=[list(range(N_CORES))],
                            ins=[a2a_in[0][:]], outs=[a2a_out1[:]],
                        )
            else:
             for _rep in range(reps):
              with tc.tile_pool(name="pt", bufs=3) as ptp, \
                 tc.tile_pool(name="nrm", bufs=3) as nrm, \
                 tc.tile_pool(name="pss", bufs=2, space="PSUM") as pss, \
                 tc.tile_pool(name="psb", bufs=1, space="PSUM") as psb, \
                 tc.tile_pool(name="pso", bufs=1, space="PSUM") as pso:
                def norm_late(st):
                    # deferred normalize tail: PE broadcast of 1/denom + the
                    # multiplies. Emitted a few kc-steps into the NEXT column
                    # so the in-order PE queue has score/PV work queued ahead
                    # and never stalls waiting on the DVE recip chain.
                    oA, oB, recA, recB, dd_, qc_ = st
                    rbA = psb.tile([64, 512], F32, tag="rbA")
                    nc.tensor.matmul(out=rbA[:], lhsT=ones_sb[0:1, 0:64],
                                     rhs=recA[:], start=True, stop=True)
                    rbB = psb.tile([64, 512], F32, tag="rbB")
                    nc.tensor.matmul(out=rbB[:], lhsT=ones_sb[0:1, 0:64],
                                     rhs=recB[:], start=True, stop=True)
                    nc.vector.tensor_tensor(out=attnv[0:64, dd_, qc_ * 512:(qc_ + 1) * 512],
                                            in0=oA[0:64, :], in1=rbA[:],
                                            op=mybir.AluOpType.mult)
                    nc.vector.tensor_tensor(out=attnv[64:128, dd_, qc_ * 512:(qc_ + 1) * 512],
                                            in0=oB[0:64, :], in1=rbB[:],
                                            op=mybir.AluOpType.mult)

                pending = None
                for dd in range(2):
                    for qc in range(4):
                        poA = pso.tile([65, 512], F32, tag="poA")
                        poB = pso.tile([65, 512], F32, tag="poB")
                        for kc in range(16):
                            pscr = pss.tile([128, 1024], F32, tag="pscr")
                            nc.tensor.matmul(out=pscr[:, 0:512],
                                             lhsT=kT_sb[0:64, dd, kc * 128:(kc + 1) * 128],
                                             rhs=qT_sb[0:64, dd, qc * 512:(qc + 1) * 512],
                                             start=True, stop=True, tile_position=(0, 0))
                            nc.tensor.matmul(out=pscr[:, 512:1024],
                                             lhsT=kT_sb[64:128, dd, kc * 128:(kc + 1) * 128],
                                             rhs=qT_sb[64:128, dd, qc * 512:(qc + 1) * 512],
                                             start=True, stop=True, tile_position=(64, 0))
                            pt = ptp.tile([128, 1024], BF16, tag="pt")
                            nc.scalar.activation(out=pt[:], in_=pscr[:],
                                                 func=mybir.ActivationFunctionType.Exp,
                                                 scale=0.125)
                            nc.tensor.matmul(out=poA[:], lhsT=v_sb[:, kc, 2 * dd, :],
                                             rhs=pt[:, 0:512],
                                             start=(kc == 0), stop=(kc == 15))
                            nc.tensor.matmul(out=poB[:], lhsT=v_sb[:, kc, 2 * dd + 1, :],
                                             rhs=pt[:, 512:1024],
                                             start=(kc == 0), stop=(kc == 15))
                            if kc == 2 and pending is not None:
                                norm_late(pending)
                                pending = None

                        # normalize head: evacuate PSUM (frees poA/poB for the
                        # next column) and take reciprocals of the ones-row sums
                        oA = nrm.tile([65, 512], F32, tag="oA")
                        nc.vector.tensor_copy(out=oA[:], in_=poA[:])
                        oB = nrm.tile([65, 512], F32, tag="oB")
                        nc.vector.tensor_copy(out=oB[:], in_=poB[:])
                        recA = nrm.tile([1, 512], BF16, tag="recA")
                        recB = nrm.tile([1, 512], BF16, tag="recB")
                        with nc.allow_low_precision(reason="1/denom feeds bf16 attnv"):
                            nc.vector.reciprocal(out=recA[:], in_=oA[64:65, :])
                            nc.vector.reciprocal(out=recB[:], in_=oB[64:65, :])
                        pending = (oA, oB, recA, recB, dd, qc)

                    # flush before the staging DMAs read attnv for this duo
                    if pending is not None:
                        norm_late(pending)
                        pending = None

                    if _rep == reps - 1:
                        # ship this duo's attention output: block d holds
                        # [128 ch, 256 query] for destination core d
                        for d in range(N_CORES):
                            if KCC == 1:
                                dst = a2a_in[0][(d * 2 + dd) * A2A_BLK:
                                                (d * 2 + dd + 1) * A2A_BLK]
                            else:
                                dst = a2a_in[dd][d * A2A_BLK:(d + 1) * A2A_BLK]
                            nc.sync.dma_start(
                                out=dst.rearrange("(p f) -> p f", p=128),
                                in_=attnv[:, dd, d * TC:(d + 1) * TC])
                        if KCC == 2:
                            nc.gpsimd.collective_compute(
                                "AllToAll", mybir.AluOpType.bypass,
                                replica_groups=[list(range(N_CORES))],
                                ins=[a2a_in[dd][:]], outs=[a2a_out[dd][:]],
                            )
                        elif KCC == 0:
                            nc.gpsimd.dma_start(
                                out=a2a_out[dd][:].rearrange("r f -> (r f)"),
                                in_=a2a_in[dd][:])
                        elif KCC == 1 and dd == 1:
                            nc.gpsimd.collective_compute(
                                "AllToAll", mybir.AluOpType.bypass,
                                replica_groups=[list(range(N_CORES))],
                                ins=[a2a_in[0][:]], outs=[a2a_out1[:]],
                            )

            # ---------------- phase 4: o_net + residual + layernorm ----
            with tc.tile_pool(name="fin", bufs=2) as fin, \
                 tc.tile_pool(name="psf", bufs=4, space="PSUM") as psf:
                # gather A2A results: att_in[p, s, b2, t], p = (h2-2s)*64 + ch
                for dd in range(2):
                    for b2 in range(2):
                        if KCC == 1:
                            src = bass.AP(
                                tensor=a2a_out1.tensor,
                                offset=a2a_out1.offset + dd * A2A_BLK + b2 * 64 * TC,
                                ap=[[TC, 64], [2 * A2A_BLK, 8], [1, TC]])
                        else:
                            src = bass.AP(
                                tensor=a2a_out[dd].tensor,
                                offset=a2a_out[dd].offset + b2 * 64 * TC,
                                ap=[[TC, 64], [A2A_BLK, 8], [1, TC]])
                        nc.sync.dma_start(out=att_in[dd * 64:(dd + 1) * 64, :, b2, :],
                                          in_=src)

                for b2 in range(2):
                    for tch in range(2):
                        chunk = b2 * 2 + tch
                        x = fin.tile([128, D], F32, tag="x")
                        for nn_ in range(2):
                            po = psf.tile([128, 512], F32, tag="po")
                            for s in range(8):
                                nc.tensor.matmul(
                                    out=po[:],
                                    lhsT=att_in[:, s, b2, tch * 128:(tch + 1) * 128],
                                    rhs=wo_sb[:, s, nn_ * 512:(nn_ + 1) * 512],
                                    start=(s == 0), stop=(s == 7))
                            nc.vector.tensor_tensor(out=x[:, nn_ * 512:(nn_ + 1) * 512],
                                                    in0=po[:],
                                                    in1=res_sb[:, chunk, nn_ * 512:(nn_ + 1) * 512],
                                                    op=mybir.AluOpType.add)
                        stats = fin.tile([128, 2, 6], F32, tag="stats")
                        for s2 in range(2):
                            nc.vector.bn_stats(out=stats[:, s2, :], in_=x[:, s2 * 512:(s2 + 1) * 512])
                        mv = fin.tile([128, 2], F32, tag="mv")
                        nc.vector.bn_aggr(out=mv[:], in_=stats[:])
                        sd = fin.tile([128, 1], F32, tag="sd")
                        nc.scalar.activation(out=sd[:], in_=mv[:, 1:2],
                                             func=mybir.ActivationFunctionType.Sqrt,
                                             bias=eps_sb[:], scale=1.0)
                        rstd = fin.tile([128, 1], F32, tag="rstd")
                        nc.vector.reciprocal(out=rstd[:], in_=sd[:])
                        y = fin.tile([128, D], F32, tag="y")
                        nc.vector.tensor_scalar(out=y[:], in0=x[:],
                                                scalar1=mv[:, 0:1], scalar2=rstd[:],
                                                op0=mybir.AluOpType.subtract,
                                                op1=mybir.AluOpType.mult)
                        yg = fin.tile([128, D], F32, tag="yg")
                        nc.gpsimd.tensor_tensor(out=yg[:], in0=y[:], in1=gb_sb[:],
                                                op=mybir.AluOpType.mult)
                        yb = fin.tile([128, D], F32, tag="yb")
                        nc.gpsimd.tensor_tensor(out=yb[:], in0=yg[:], in1=bb_sb[:],
                                                op=mybir.AluOpType.add)
                        nc.sync.dma_start(
                            out=out[chunk * 128:(chunk + 1) * 128, :], in_=yb[:])

    nc.finalize()
    return nc


def _get_program(reps=1):
    if reps not in _prog_cache:
        _prog_cache[reps] = _build_program(reps)
    return _prog_cache[reps]


def _prep_inputs(inp, W_qkv, b_qkv, W_o, gamma, beta):
    """Build the 8 per-core input dicts (host-side)."""
    f32 = np.float32
    inp = np.asarray(inp, f32)
    W_qkv = np.asarray(W_qkv, f32)
    b_qkv = np.asarray(b_qkv, f32)
    W_o = np.asarray(W_o, f32)
    gamma = np.asarray(gamma, f32).reshape(1, D)
    beta = np.asarray(beta, f32).reshape(1, D)

    woT = np.ascontiguousarray(
        W_o.T.reshape(8, 128, 1024).transpose(1, 0, 2)).astype(nbf16)
    ones = np.ones((1, T), nbf16)

    xT_b = []
    for b in range(B):
        xT_b.append(np.ascontiguousarray(
            inp[b].T.reshape(8, 128, T).transpose(1, 0, 2)).astype(nbf16))

    in_maps = []
    for c in range(N_CORES):
        bc = c // 4
        r0 = 4 * (c % 4) * 64            # first channel row of this core's heads
        rows = slice(r0, r0 + 256)
        wqT = np.ascontiguousarray(
            W_qkv[0:1024][rows].T.reshape(8, 128, 256).transpose(1, 0, 2)).astype(nbf16)
        wkT = np.ascontiguousarray(
            W_qkv[1024:2048][rows].T.reshape(8, 128, 256).transpose(1, 0, 2)).astype(nbf16)
        wvT = np.ascontiguousarray(
            W_qkv[2048:3072][rows].T.reshape(8, 128, 256).transpose(1, 0, 2)).astype(nbf16)
        sl = slice(c * TC, (c + 1) * TC)
        x_res = np.concatenate([inp[0, sl, :], inp[1, sl, :]], axis=0)
        in_maps.append({
            "xT": xT_b[bc],
            "inp_res": np.ascontiguousarray(x_res),
            "wqT": wqT, "wkT": wkT, "wvT": wvT, "woT": woT,
            "b_q": b_qkv[0:1024][rows].reshape(1, 256).astype(nbf16),
            "b_k": b_qkv[1024:2048][rows].reshape(1, 256).astype(nbf16),
            "b_v": b_qkv[2048:3072][rows].reshape(1, 256).astype(nbf16),
            "onesd": ones, "gamma": gamma, "beta": beta,
        })
    return in_maps


def _assemble(results):
    out = np.empty((B, T, D), np.float32)
    for c in range(N_CORES):
        o = results[c]["out"]
        sl = slice(c * TC, (c + 1) * TC)
        out[0, sl, :] = o[0:TC, :]
        out[1, sl, :] = o[TC:NTOK, :]
    return out


def kernel(inp, W_qkv, b_qkv, W_o, gamma, beta):
    nc = _get_program()
    in_maps = _prep_inputs(inp, W_qkv, b_qkv, W_o, gamma, beta)
    res = run_bass_kernel_spmd(nc, in_maps, core_ids=list(range(N_CORES)))
    return _assemble(res.results)


if __name__ == "__main__":
    rng = np.random.RandomState(0)
    inp = rng.randn(B, T, D).astype(np.float32)
    W_qkv = (rng.randn(3 * H * DH, D) * D ** -0.5).astype(np.float32)
    b_qkv = (rng.randn(3 * H * DH) * 0.02).astype(np.float32)
    W_o = (rng.randn(D, H * DH) * (H * DH) ** -0.5).astype(np.float32)
    gamma = np.ones(D, np.float32)
    beta = np.zeros(D, np.float32)
    out = kernel(inp=inp, W_qkv=W_qkv, b_qkv=b_qkv, W_o=W_o, gamma=gamma, beta=beta)
    print("out", out.shape, out.dtype, np.abs(out).mean())
